# revision 10
# baseline (speedup 1.0000x reference)
"""Trainium2 Bass kernel for nn_ChannelSegment (differential-attention MoE).

Sharding: 8 cores = 4 channels x 2 batches; core i handles (b, n) = (i//4, i%4).

v2 design notes (vs the v1 baseline at 308us):
- ALL matmuls bf16 (v1's float32r tiles lowered to fp32_mode=HIGH matmuls at
  ~3.6 cycles/row and the PE sat at HAM K=4/8 for 70% of the kernel).
- Score matmuls (K=32) packed 2-at-a-time into PE row groups via
  tile_position; exp activations batched 2 streams per call from a
  [128,2,512] PSUM tile (ACT is the #2 engine at ~1 elem/cycle/lane).
- PV accumulators live in one [65,4,512] PSUM tile (4 banks); softmax
  denominators (ones-row of v_aug) are consumed directly from PSUM by
  reciprocal/mult DVE ops - no drain copies of the accumulators at all.
- Epilogue in ratio form u = o1 - lam*(den1/den2) o2 (the 1/den1 scale
  cancels in the diff-RMS), one K=1 broadcast matmul per head.
- Squares and causal tri-masking on GPSIMD (otherwise idle).
- MM2/out-proj computed token-major so the final RMS over CW=512 becomes a
  free-dim tensor_tensor_reduce and the residual+scale are per-partition ops.
- rsqrt computed as exp(-0.5*ln(x)) so the whole middle of the kernel stays
  on the natural_log_exp activation table set (no table thrash with Exp).
"""
import os
import sys

sys.path.insert(0, "/opt/trn_rl_repo")

import numpy as np
import ml_dtypes

from concourse import bacc
import concourse.tile as tile
from concourse import mybir
from concourse.bass_utils import run_bass_kernel_spmd

N_CH, CW, H, D, D2 = 4, 512, 8, 64, 32
L, B = 1024, 2
EPS = 1e-6
LAM0 = 0.2
SCALE = float(1.0 / np.sqrt(np.float32(D2)))

F32 = mybir.dt.float32
BF16 = mybir.dt.bfloat16
AF = mybir.ActivationFunctionType
OP = mybir.AluOpType

_cache = {}


def _build(ph=6):
    from contextlib import ExitStack

    nc = bacc.Bacc("TRN2", target_bir_lowering=False, num_devices=8)

    dp = nc.declare_dram_parameter
    hT_d = dp("hT", [CW, L], BF16, isOutput=False)
    hw_d = dp("hw", [8, 128, CW], F32, isOutput=False)
    wqk_d = dp("wqk", [CW, 2 * CW], BF16, isOutput=False)
    wv_d = dp("wv", [CW, CW], BF16, isOutput=False)
    wout_d = dp("wout", [8, 64, CW], BF16, isOutput=False)
    bqk_d = dp("bqk", [8, 128, 1], F32, isOutput=False)
    bvr_d = dp("bvr", [CW], F32, isOutput=False)
    boutr_d = dp("boutr", [CW], F32, isOutput=False)
    wnwr_d = dp("wnwr", [CW], BF16, isOutput=False)
    qmul_d = dp("qmul", [4, 128, 1], F32, isOutput=False)
    whs_d = dp("whs", [64, 1], F32, isOutput=False)
    lam64_d = dp("lam64", [1, 64], BF16, isOutput=False)
    ones64_d = dp("ones64", [64, 1], BF16, isOutput=False)
    o1x64_d = dp("o1x64", [1, 64], BF16, isOutput=False)
    e4_d = dp("e4", [128, 4], BF16, isOutput=False)
    b4_d = dp("b4", [128, 128], BF16, isOutput=False)
    tri2_d = dp("tri2", [128, 256], BF16, isOutput=False)
    y_d = dp("y", [8, 128, CW], F32, isOutput=True)

    with tile.TileContext(nc) as tc:
        est = ExitStack()
        est.enter_context(nc.allow_low_precision(reason="bf16 matmul pipeline"))
        persist = est.enter_context(tc.tile_pool(name="persist", bufs=1))
        # PSUM: big_ps 2 bufs x 2 banks + pv_ps 1 buf x 4 banks = 8 banks
        big_ps = est.enter_context(tc.tile_pool(name="big_ps", bufs=2, space="PSUM"))
        pv_ps = est.enter_context(tc.tile_pool(name="pv_ps", bufs=1, space="PSUM"))
        sqp = est.enter_context(tc.tile_pool(name="sqp", bufs=4))
        pp = est.enter_context(tc.tile_pool(name="pp", bufs=3))
        up = est.enter_context(tc.tile_pool(name="up", bufs=2))
        mp = est.enter_context(tc.tile_pool(name="mp", bufs=2))
        rowp = est.enter_context(tc.tile_pool(name="rowp", bufs=2))
        rtp = est.enter_context(tc.tile_pool(name="rtp", bufs=2))
        rallp = est.enter_context(tc.tile_pool(name="rallp", bufs=2))
        t1p = est.enter_context(tc.tile_pool(name="t1p", bufs=3))
        yp = est.enter_context(tc.tile_pool(name="yp", bufs=3))
        scr = est.enter_context(tc.tile_pool(name="scr", bufs=2))

        dma = nc.sync.dma_start

        # ---- constants / inputs ----
        hT = [persist.tile([128, L], BF16, tag=f"hT{k}", name=f"hT{k}") for k in range(4)]
        wqk = [persist.tile([128, 2 * CW], BF16, tag=f"wq{k}", name=f"wq{k}") for k in range(4)]
        for k in range(4):
            dma(out=hT[k], in_=hT_d[128 * k : 128 * (k + 1), :])
            dma(out=wqk[k], in_=wqk_d[128 * k : 128 * (k + 1), :])
        wv = [persist.tile([128, CW], BF16, tag=f"wv{k}", name=f"wv{k}") for k in range(4)]
        for k in range(4):
            dma(out=wv[k], in_=wv_d[128 * k : 128 * (k + 1), :])
        wout_c = [persist.tile([64, CW], BF16, tag=f"wo{k}", name=f"wo{k}") for k in range(8)]
        for k in range(8):
            dma(out=wout_c[k], in_=wout_d[k])
        hw = [persist.tile([128, CW], F32, tag=f"hw{t}", name=f"hw{t}") for t in range(8)]
        for t in range(8):
            dma(out=hw[t], in_=hw_d[t])

        bqk = [persist.tile([128, 1], F32, tag=f"bqk{j}", name=f"bqk{j}") for j in range(8)]
        for j in range(8):
            dma(out=bqk[j], in_=bqk_d[j])
        bv_bc = persist.tile([128, CW], F32, tag="bv_bc", name="bv_bc")
        dma(out=bv_bc, in_=bvr_d[:].partition_broadcast(128))
        bout_bc = persist.tile([128, CW], F32, tag="bout_bc", name="bout_bc")
        dma(out=bout_bc, in_=boutr_d[:].partition_broadcast(128))
        wnw_bc = persist.tile([128, CW], BF16, tag="wnw_bc", name="wnw_bc")
        dma(out=wnw_bc, in_=wnwr_d[:].partition_broadcast(128))
        qmul = [persist.tile([128, 1], F32, tag=f"qm{j}", name=f"qm{j}") for j in range(4)]
        for j in range(4):
            dma(out=qmul[j], in_=qmul_d[j])
        whs = persist.tile([64, 1], F32, tag="whs", name="whs")
        dma(out=whs, in_=whs_d[:])
        lam64 = persist.tile([1, 64], BF16, tag="lam64", name="lam64")
        dma(out=lam64, in_=lam64_d[:])
        ones64 = persist.tile([64, 1], BF16, tag="ones64", name="ones64")
        dma(out=ones64, in_=ones64_d[:])
        o1x64 = persist.tile([1, 64], BF16, tag="o1x64", name="o1x64")
        dma(out=o1x64, in_=o1x64_d[:])
        e4all = persist.tile([128, 4], BF16, tag="e4all", name="e4all")
        dma(out=e4all, in_=e4_d[:])
        b4all = persist.tile([128, 128], BF16, tag="b4all", name="b4all")
        dma(out=b4all, in_=b4_d[:])
        tri2 = persist.tile([128, 2, 128], BF16, tag="tri2", name="tri2")
        dma(out=tri2, in_=tri2_d[:].rearrange("p (g x) -> p g x", x=128))
        eps_sb = persist.tile([128, 1], F32, tag="eps_sb", name="eps_sb")
        nc.vector.memset(eps_sb, EPS)

        # ---- P1: qkT[j] = silu(wqk.T @ hT + bqk), bf16 [128, 1024] ----
        qkT = [persist.tile([128, L], BF16, tag=f"qkT{j}", name=f"qkT{j}") for j in range(8)]
        for j in range(8 if ph >= 1 else 0):
            ps = big_ps.tile([128, 2, 512], F32, tag="big", name=f"mm1a{j}")
            for k in range(4):
                for c in range(2):
                    nc.tensor.matmul(
                        ps[:, c, :],
                        wqk[k][:, 128 * j : 128 * (j + 1)],
                        hT[k][:, 512 * c : 512 * (c + 1)],
                        start=(k == 0),
                        stop=(k == 3),
                    )
            nc.scalar.activation(
                out=qkT[j].rearrange("p (c x) -> p c x", x=512),
                in_=ps,
                func=AF.Silu,
                bias=bqk[j],
            )

        # ---- P2: v_aug[t][:, h, 0:64] = silu(h @ wv + bv); ones in col 64 ----
        v_aug = [persist.tile([128, 8, 65], BF16, tag=f"va{t}", name=f"va{t}") for t in range(8)]
        for t in range(8 if ph >= 2 else 0):
            nc.vector.memset(v_aug[t][:, :, 64:65], 1.0)
            ps = big_ps.tile([128, 512], F32, tag="big", name=f"mm1b{t}")
            for k in range(4):
                nc.tensor.matmul(
                    ps,
                    hT[k][:, 128 * t : 128 * (t + 1)],
                    wv[k],
                    start=(k == 0),
                    stop=(k == 3),
                )
            t1 = t1p.tile([128, 512], BF16, tag="t1", name=f"vt{t}")
            nc.vector.tensor_add(out=t1, in0=ps, in1=bv_bc)
            nc.scalar.activation(
                out=v_aug[t][:, :, 0:64],
                in_=t1.rearrange("p (h d) -> p h d", d=64),
                func=AF.Silu,
            )

        # ---- P3: group-RMS of q/k; fold wq*wk into q ----
        # squares on gpsimd; col-tiled e4 matmuls collect 4 j-tiles' group
        # sums into one PSUM bank; rsqrt via ln->exp; b4all broadcast matmul.
        for half in range(2 if ph >= 3 else 0):  # 0: q tiles j=0..3, 1: k tiles j=4..7
            for c in range(2):
                msq = big_ps.tile([128, 512], F32, tag="big", name=f"msq{half}{c}")
                nc.vector.memset(msq, 1.0)
                sqs = []
                for jj in range(4):
                    j = 4 * half + jj
                    sq = sqp.tile([128, 512], BF16, tag="sq", name=f"sq{j}{c}")
                    nc.gpsimd.tensor_mul(
                        out=sq,
                        in0=qkT[j][:, 512 * c : 512 * (c + 1)],
                        in1=qkT[j][:, 512 * c : 512 * (c + 1)],
                    )
                    sqs.append(sq)
                for jj in range(4):
                    nc.tensor.matmul(
                        msq[32 * jj : 32 * jj + 4, :],
                        e4all,
                        sqs[jj],
                        start=True,
                        stop=True,
                        tile_position=(0, 32 * jj),
                    )
                # rall = rsqrt(msq/32 + eps) = exp(-0.5 * ln(msq/32 + eps))
                lnm = rallp.tile([128, 512], F32, tag="lnm", name=f"lnm{half}{c}")
                nc.scalar.activation(
                    out=lnm, in_=msq, func=AF.Ln, scale=1.0 / 32.0, bias=eps_sb
                )
                rall = rallp.tile([128, 512], BF16, tag="rall", name=f"rall{half}{c}")
                nc.scalar.activation(out=rall, in_=lnm, func=AF.Exp, scale=-0.5)
                for jj in range(4):
                    j = 4 * half + jj
                    rbc = big_ps.tile([128, 512], F32, tag="big", name=f"rbc{j}{c}")
                    nc.tensor.matmul(
                        rbc,
                        b4all[32 * jj : 32 * jj + 4, :],
                        rall[32 * jj : 32 * jj + 4, :],
                        start=True,
                        stop=True,
                        tile_position=(32 * jj, 0),
                    )
                    if half == 0:
                        nc.vector.scalar_tensor_tensor(
                            out=qkT[j][:, 512 * c : 512 * (c + 1)],
                            in0=qkT[j][:, 512 * c : 512 * (c + 1)],
                            scalar=qmul[j],
                            in1=rbc,
                            op0=OP.mult,
                            op1=OP.mult,
                        )
                    else:
                        nc.vector.tensor_mul(
                            out=qkT[j][:, 512 * c : 512 * (c + 1)],
                            in0=qkT[j][:, 512 * c : 512 * (c + 1)],
                            in1=rbc,
                        )

        # ---- P4/P5: attention per head-pair jq, token half c ----
        # diffn stored per (head, c-half): 8 tiles [64, 1024] bf16
        diffn = [persist.tile([64, L], BF16, tag=f"dn{q}", name=f"dn{q}") for q in range(8)]
        for q in range(8):
            nc.vector.memset(diffn[q], 0.001)
        for jq in range(4 if ph >= 4 else 0):
            jk = 4 + jq
            for c in range(2):
                nt = 4 * c + 4
                pv = pv_ps.tile([65, 4, 512], F32, tag="pv", name=f"pv{jq}{c}")
                ptiles = {}
                for t in range(nt):
                    off = max(0, 128 * (t - 4 * c))
                    w = 512 - off
                    for hh in range(2):
                        h = 2 * jq + hh
                        sc = big_ps.tile([128, 2, 512], F32, tag="big", name=f"sc{jq}{c}{t}{hh}")
                        for br in range(2):
                            band = 64 * hh + 32 * br
                            nc.tensor.matmul(
                                sc[:, br, 0:w],
                                qkT[jk][band : band + 32, 128 * t : 128 * (t + 1)],
                                qkT[jq][band : band + 32, 512 * c + off : 512 * (c + 1)],
                                start=True,
                                stop=True,
                                tile_position=(band, 0),
                            )
                        p = pp.tile([128, 2, 512], BF16, tag="p", name=f"p{jq}{c}{t}{hh}")
                        nc.scalar.activation(
                            out=p[:, :, 0:w], in_=sc[:, :, 0:w], func=AF.Exp, scale=SCALE
                        )
                        if t >= 4 * c:
                            nc.gpsimd.tensor_mul(
                                out=p[:, :, 0:128], in0=p[:, :, 0:128], in1=tri2
                            )
                        for br in range(2):
                            nc.tensor.matmul(
                                pv[:, 2 * hh + br, off:512],
                                v_aug[t][:, h, :],
                                p[:, br, 0:w],
                                start=(t == 0),
                                stop=(t == nt - 1),
                            )

                if ph < 5:
                    continue
                # ---- epilogue: u = o1 - lam*(den1/den2) o2, diff-RMS ----
                # dens: pv[64, q, :]; br0 = branch1 (den1), br1 = branch2 (den2)
                rr = rowp.tile([1, 2, 512], BF16, tag="rr", name=f"rr{jq}{c}")
                nc.vector.reciprocal(out=rr, in_=pv[64:65, 1:4:2, :])
                lr = rowp.tile([1, 2, 512], BF16, tag="lr", name=f"lr{jq}{c}")
                nc.vector.tensor_mul(out=lr, in0=pv[64:65, 0:4:2, :], in1=rr)
                rbc = big_ps.tile([128, 2, 512], F32, tag="big", name=f"rbq{jq}{c}")
                for hh in range(2):
                    nc.tensor.matmul(
                        rbc[0:64, hh, :],
                        lam64,
                        lr[:, hh, :],
                        start=True,
                        stop=True,
                        tile_position=(0, 0),
                    )
                rbs = mp.tile([64, 2, 512], BF16, tag="rbs", name=f"rbs{jq}{c}")
                nc.vector.tensor_copy(out=rbs, in_=rbc[0:64, :, :])
                u = up.tile([64, 2, 512], BF16, tag="u", name=f"u{jq}{c}")
                m = mp.tile([64, 2, 512], BF16, tag="m", name=f"m{jq}{c}")
                for hh in range(2):
                    nc.vector.tensor_mul(
                        out=m[:, hh, :], in0=pv[0:64, 2 * hh + 1, :], in1=rbs[:, hh, :]
                    )
                    nc.vector.tensor_sub(
                        out=u[:, hh, :], in0=pv[0:64, 2 * hh, :], in1=m[:, hh, :]
                    )
                usq = up.tile([64, 2, 512], BF16, tag="usq", name=f"usq{jq}{c}")
                nc.vector.tensor_mul(out=usq, in0=u, in1=u)
                dm = big_ps.tile([1, 2, 512], F32, tag="big", name=f"dm{jq}{c}")
                for hh in range(2):
                    nc.tensor.matmul(
                        dm[:, hh, :],
                        ones64,
                        usq[:, hh, :],
                        start=True,
                        stop=True,
                        tile_position=(0, 0),
                    )
                # rt = rsqrt(dm/64 + eps) via ln->exp
                lnt = rtp.tile([1, 2, 512], F32, tag="lnt", name=f"lnt{jq}{c}")
                nc.scalar.activation(
                    out=lnt, in_=dm, func=AF.Ln, scale=1.0 / 64.0, bias=eps_sb[0:1, :]
                )
                rt = rtp.tile([1, 2, 512], BF16, tag="rt", name=f"rt{jq}{c}")
                nc.scalar.activation(out=rt, in_=lnt, func=AF.Exp, scale=-0.5)
                rtbc = big_ps.tile([64, 2, 512], F32, tag="big", name=f"rtb{jq}{c}")
                for hh in range(2):
                    nc.tensor.matmul(
                        rtbc[:, hh, :],
                        o1x64,
                        rt[:, hh, :],
                        start=True,
                        stop=True,
                        tile_position=(0, 0),
                    )
                for hh in range(2):
                    nc.vector.scalar_tensor_tensor(
                        out=diffn[2 * jq + hh][:, 512 * c : 512 * (c + 1)],
                        in0=u[:, hh, :],
                        scalar=whs,
                        in1=rtbc[:, hh, :],
                        op0=OP.mult,
                        op1=OP.mult,
                    )

        # ---- P6: out-proj token-major + final RMS + residual ----
        attn = [persist.tile([128, 512], BF16, tag=f"at{lt}", name=f"at{lt}") for lt in range(8)]
        for lt in range(8):
            ps = big_ps.tile([128, 512], F32, tag="big", name=f"mm2{lt}")
            for ch in range(8):
                nc.tensor.matmul(
                    ps,
                    diffn[ch][:, 128 * lt : 128 * (lt + 1)],
                    wout_c[ch],
                    start=(ch == 0),
                    stop=(ch == 7),
                )
            t1 = t1p.tile([128, 512], BF16, tag="t1", name=f"ot{lt}")
            nc.vector.tensor_add(out=t1, in0=ps, in1=bout_bc)
            nc.scalar.activation(out=attn[lt], in_=t1, func=AF.Silu)
        msqf = persist.tile([128, 8], F32, tag="msqf", name="msqf")
        for lt in range(8):
            s = scr.tile([128, 512], BF16, tag="scr", name=f"sc2{lt}")
            nc.vector.tensor_mul(out=s, in0=attn[lt], in1=attn[lt])
            nc.vector.reduce_sum(out=msqf[:, lt : lt + 1], in_=s, axis=mybir.AxisListType.X)
        lnf = persist.tile([128, 8], F32, tag="lnf", name="lnf")
        nc.scalar.activation(
            out=lnf, in_=msqf, func=AF.Ln, scale=1.0 / 512.0, bias=eps_sb
        )
        rf = persist.tile([128, 8], F32, tag="rf", name="rf")
        nc.scalar.activation(out=rf, in_=lnf, func=AF.Exp, scale=-0.5)
        for lt in range(8):
            t2 = yp.tile([128, 512], BF16, tag="t2", name=f"t2{lt}")
            nc.vector.scalar_tensor_tensor(
                out=t2,
                in0=attn[lt],
                scalar=rf[:, lt : lt + 1],
                in1=wnw_bc,
                op0=OP.mult,
                op1=OP.mult,
            )
            y = yp.tile([128, 512], F32, tag="y", name=f"y{lt}")
            nc.vector.tensor_add(out=y, in0=t2, in1=hw[lt])
            dma(out=y_d[lt], in_=y)
        est.close()

    nc.compile()
    return nc


def kernel(x, routing_weights, Wqkv, bqkv, Wout, bout, lq1, lk1, lq2, lk2, wq, wk, wh, wn):
    if "nc" not in _cache:
        _cache["nc"] = _build(int(os.environ.get("KPH", "6")))
    nc = _cache["nc"]

    x = np.asarray(x, np.float32)
    routing_weights = np.asarray(routing_weights, np.float32)
    Wqkv = np.asarray(Wqkv, np.float32)
    bqkv = np.asarray(bqkv, np.float32)
    Wout = np.asarray(Wout, np.float32)
    bout = np.asarray(bout, np.float32)
    lq1, lk1 = np.asarray(lq1, np.float32), np.asarray(lk1, np.float32)
    lq2, lk2 = np.asarray(lq2, np.float32), np.asarray(lk2, np.float32)
    wq, wk = np.asarray(wq, np.float32), np.asarray(wk, np.float32)
    wh, wn = np.asarray(wh, np.float32), np.asarray(wn, np.float32)

    bf = ml_dtypes.bfloat16
    tri = np.triu(np.ones((128, 128), np.float32))
    tri2 = np.concatenate([tri, tri], axis=1).astype(bf)  # [128, 256]
    e4 = np.zeros((128, 4), np.float32)
    for g in range(4):
        e4[32 * g : 32 * (g + 1), g] = 1.0
    b4 = np.zeros((128, 128), np.float32)
    for jj in range(4):
        for g in range(4):
            b4[32 * jj + g, 32 * g : 32 * (g + 1)] = 1.0
    ones64 = np.ones((64, 1), np.float32).astype(bf)
    o1x64 = np.ones((1, 64), np.float32).astype(bf)

    in_maps = []
    for i in range(8):
        b, n = i // 4, i % 4
        w = float(routing_weights[b, n])
        lam = float(
            np.exp(np.dot(lq1[n], lk1[n]).astype(np.float32))
            - np.exp(np.dot(lq2[n], lk2[n]).astype(np.float32))
            + np.float32(LAM0)
        )
        wqwk = (wq[n] * wk[n]).astype(np.float32)  # [32]
        xs = x[b, :, CW * n : CW * (n + 1)]  # [L, CW]
        in_maps.append(
            dict(
                hT=np.ascontiguousarray(xs.T).astype(bf),
                hw=np.ascontiguousarray((xs * w).reshape(8, 128, CW)),
                wqk=np.ascontiguousarray(Wqkv[n][:, : 2 * CW]).astype(bf),
                wv=np.ascontiguousarray(Wqkv[n][:, 2 * CW :]).astype(bf),
                wout=np.ascontiguousarray(Wout[n].reshape(8, 64, CW)).astype(bf),
                bqk=np.ascontiguousarray(bqkv[n][: 2 * CW].reshape(8, 128, 1)),
                bvr=np.ascontiguousarray(bqkv[n][2 * CW :]),
                boutr=np.ascontiguousarray(bout[n]),
                wnwr=np.ascontiguousarray(wn[n] * w).astype(bf),
                qmul=np.ascontiguousarray(np.tile(wqwk, 16).reshape(4, 128, 1)),
                whs=np.full((64, 1), 1.0, np.float32) * np.tile(wh[n], 1).reshape(64, 1) * (1.0 - LAM0),
                lam64=np.full((1, 64), lam, np.float32).astype(bf),
                ones64=ones64,
                o1x64=o1x64,
                e4=e4.astype(bf),
                b4=b4.astype(bf),
                tri2=tri2,
            )
        )

    prof_dir = os.environ.get("KERNEL_PROFILE_DIR")
    if prof_dir:
        res = run_bass_kernel_spmd(
            nc, in_maps, list(range(8)), trace=True, tmpdir=prof_dir
        )
        _cache["exec_time_ns"] = res.exec_time_ns
    else:
        res = run_bass_kernel_spmd(nc, in_maps, list(range(8)))

    out = np.empty((B, L, N_CH * CW), np.float32)
    for i in range(8):
        b, n = i // 4, i % 4
        out[b, :, CW * n : CW * (n + 1)] = res.results[i]["y"].reshape(L, CW)
    return out
